# revision 23
# baseline (speedup 1.0000x reference)
"""NF4-quantized LoRA linear layer on 8 Trainium2 NeuronCores.

Computation (reference):
    w = NF4_TABLE[w_codes] * w_scales[block-expanded]        # [O, I]
    out = x @ w.T + (alpha/rank) * (x @ lora_a.T) @ lora_b.T # [B, S, O]

Strategy:
  - Tensor-parallel split of the output dim across 8 cores (O_SH = 512 each).
    Every core sees all of x; no collectives; host concatenates outputs.
  - LoRA folded into the weights per i-tile on the PE (la.T @ lb); those
    matmuls plus a dummy burst keep the PE busy from t=0 so the HAM clock
    gate is fully open (2.4 GHz) before the real matmuls start.
  - NF4 dequant: 7-term approximate chain (f16 table err 1.5e-3, end-to-end
    err ~3.5e-3 vs the 2e-2 gate): linear+step on DVE tensor_scalar, 5 relu
    ramps on ACT, 6 DVE combines, then *scales and +lora.
  - m-loop phase 1 covers i-tiles 0-11 with M BLOCKED 8-wide: each block of
    8 m-tiles keeps its 4 psum pair-tiles open across six 2-i-tile chunks,
    consuming dequant output just-in-time.  No mid-phase partial evacuation
    or re-add exists at all; each block is evacuated once to a bf16 SBUF
    partial.  Phase 2 (i-tiles 12-31) streams the remaining contraction and
    adds the partial on evacuation.
  - Dequant DMA/compute for later macros is pumped through the block loop
    so no engine FIFO head-blocks; block evacuations live on ACT only,
    phase-2 evacuation adds on DVE, output DMA on the scalar queue.
"""

import numpy as np
import ml_dtypes

import concourse.mybir as mybir
import concourse.tile as tile
from concourse import bacc
from concourse.bass_utils import run_bass_kernel_spmd

B, S, I, O, R, BLK = 4, 2048, 4096, 4096, 16, 64
M = B * S                      # 8192 token rows
N_CORES = 8
O_SH = O // N_CORES            # 512 output cols per core
IT = I // 128                  # 32 contraction tiles
MT = M // 128                  # 64 row tiles
NPAIR = MT // 2                # 32 psum pair-tiles per phase
LORA_SCALE = 2.0               # alpha / rank

# dequant macros: six 2-i-tile chunks feed phase 1 just-in-time, then five
# 4-i-tile macros for phase 2
MACROS = [2, 2, 2, 2, 2, 2, 4, 4, 4, 4, 4]
AB_IT = 12                     # i-tiles covered by phase 1 (macros 0-5)
GP_TAIL_FROM = 6               # macros >= this run the chain tail on GPSIMD
N_WARM = 14                    # dummy warm-up matmuls
NBLK = 8                       # m-blocks in phase 1 (8 m-tiles each)

# NF4 chains: t(c) ~= a + b*c + sum_j g_j*relu(c - v_j) + d*[c>=13.5]
# CH7: 5 ramps, f16 table err 1.45e-3 -- used for phase-2 macros.
# CH5: 3 ramps, f16 table err 9.3e-3 -- used for the startup-critical
# phase-1 macros (12/32 of the contraction; total output err ~6.5e-3
# vs the 2e-2 gate).
CH7 = dict(
    a=-0.9999999999955771, b=0.3037613463764206,
    d=-0.11607743835394424, u=13.5,
    ramps=[
        (0.17424857616421482, 12.890314243043882),
        (-0.0147269920683398, 6.461280539039212),
        (-0.17365163565386407, 1.2363687528522225),
        (0.020825906737021872, 10.455589664724952),
        (-0.033414218483025136, 3.450174298600788),
    ])
CH5 = dict(
    a=-1.0000000000315237, b=0.3038071989637578,
    d=-0.11670333147042945, u=13.5,
    ramps=[
        (0.18864440149390185, 12.775812349363168),
        (-0.041756800433337744, 3.7380006069052687),
        (-0.17365163624795468, 1.2386672442106303),
    ])
N_BIAS = len(CH7["ramps"]) + len(CH5["ramps"])

F16 = mybir.dt.float16
BF16 = mybir.dt.bfloat16
F32 = mybir.dt.float32
ALU = mybir.AluOpType
ACTF = mybir.ActivationFunctionType

BF16_NP = ml_dtypes.bfloat16


def _macro_ranges():
    out, lo = [], 0
    for n in MACROS:
        out.append((lo, lo + n))
        lo += n
    return out


def _build_nc():
    nc = bacc.Bacc("TRN2", target_bir_lowering=False, debug=False,
                   num_devices=N_CORES)

    xt = nc.dram_tensor("xt", [128, MT, IT, 128], BF16, kind="ExternalInput")
    codes = nc.dram_tensor("codes", [I, O_SH], F16, kind="ExternalInput")
    scales = nc.dram_tensor("scales", [I, O_SH], F16, kind="ExternalInput")
    la = nc.dram_tensor("la", [R, I], BF16, kind="ExternalInput")
    lb = nc.dram_tensor("lb", [R, O_SH], BF16, kind="ExternalInput")
    out = nc.dram_tensor("out", [M, O_SH], F32, kind="ExternalOutput")

    codes_r = codes.ap().rearrange("(t p) o -> p t o", p=128)
    scales_r = scales.ap().rearrange("(t p) o -> p t o", p=128)
    mranges = _macro_ranges()

    with tile.TileContext(nc) as tc:
        with (
            tc.tile_pool(name="wpool", bufs=1) as wpool,
            tc.tile_pool(name="wlab", bufs=6) as wlab,
            tc.tile_pool(name="wlc", bufs=2) as wlc,
            tc.tile_pool(name="dqio", bufs=2) as dqio,
            tc.tile_pool(name="dq", bufs=2) as dq,
            tc.tile_pool(name="xpool", bufs=3) as xpool,
            tc.tile_pool(name="cpool", bufs=1) as cpool,
            tc.tile_pool(name="opool", bufs=3) as opool,
            tc.tile_pool(name="ps", bufs=4, space="PSUM") as pp,
        ):
            # ---- constants ----
            la_sb = cpool.tile([R, I], BF16, tag="la")
            nc.gpsimd.dma_start(la_sb[:], la.ap())
            lb_sb = cpool.tile([R, O_SH], BF16, tag="lb")
            nc.gpsimd.dma_start(lb_sb[:], lb.ap())
            biases = cpool.tile([128, N_BIAS], F32, tag="bias")
            for j, (g, v) in enumerate(CH7["ramps"] + CH5["ramps"]):
                nc.vector.memset(biases[:, j:j + 1], -abs(g) * v)
            # SBUF bf16 partial accumulator [128, MT*512]
            pa = cpool.tile([128, MT * O_SH], BF16, tag="pa")

            # ---- wl (lora fold) + dummy warm-up on the PE ----
            wl_tiles = {}

            def emit_wl(mi):
                it_lo, it_hi = mranges[mi]
                nt = it_hi - it_lo
                pool = wlab if mi < 6 else wlc
                wl = pool.tile([128, nt * O_SH], F16, tag="wl")
                j = 0
                cnt = 0
                while j < nt:
                    k = min(2, nt - j)
                    pl = pp.tile([128, 2 * O_SH], F32, tag="po")
                    for h in range(k):
                        it = it_lo + j + h
                        nc.tensor.matmul(
                            pl[:, h * O_SH:(h + 1) * O_SH],
                            la_sb[:, it * 128:(it + 1) * 128], lb_sb[:],
                            start=True, stop=True,
                        )
                    dst = wl[:, j * O_SH:(j + k) * O_SH]
                    if cnt % 2 == 0:
                        nc.scalar.copy(dst, pl[:, :k * O_SH])
                    else:
                        nc.vector.tensor_copy(dst, pl[:, :k * O_SH])
                    cnt += 1
                    j += k
                wl_tiles[mi] = wl

            # ---- dequant ----
            w_aps = {}
            slots = {}

            def emit_macro_dma(mi):
                it_lo, it_hi = mranges[mi]
                nt = it_hi - it_lo
                fd = nt * O_SH
                ct = dqio.tile([128, fd], F16, tag="ct")
                nc.gpsimd.dma_start(
                    ct[:].rearrange("p (t o) -> p t o", t=nt),
                    codes_r[:, it_lo:it_hi, :],
                )
                st = dqio.tile([128, fd], F16, tag="st")
                nc.gpsimd.dma_start(
                    st[:].rearrange("p (t o) -> p t o", t=nt),
                    scales_r[:, it_lo:it_hi, :],
                )
                slots[mi] = (ct, st)

            def chain_ops(mi):
                it_lo, it_hi = mranges[mi]
                nt = it_hi - it_lo
                fd = nt * O_SH
                tail_eng = nc.gpsimd if mi >= GP_TAIL_FROM else nc.vector
                ch = CH5 if mi < 6 else CH7
                boff = len(CH7["ramps"]) if mi < 6 else 0
                state = {}
                ops = []

                def op_lin():
                    ct, _ = slots[mi]
                    acc = dq.tile([128, fd], F16, tag="acc")
                    nc.vector.tensor_scalar(
                        acc[:], ct[:], ch["b"], ch["a"],
                        op0=ALU.mult, op1=ALU.add)
                    state["acc"] = acc
                ops.append(op_lin)
                for j, (g, v) in enumerate(ch["ramps"]):
                    def op_ramp(j=j, g=g):
                        ct, _ = slots[mi]
                        r = dq.tile([128, fd], F16, tag="rmp")
                        nc.scalar.activation(
                            r[:], ct[:], ACTF.Relu,
                            bias=biases[:, boff + j:boff + j + 1],
                            scale=abs(g))
                        state["r"] = r
                    ops.append(op_ramp)

                    def op_comb(g=g):
                        acc = state["acc"]
                        nc.vector.tensor_tensor(
                            acc[:], acc[:], state["r"][:],
                            op=ALU.add if g > 0 else ALU.subtract)
                    ops.append(op_comb)

                def op_step():
                    ct, _ = slots[mi]
                    stp = dq.tile([128, fd], F16, tag="rmp")
                    nc.vector.tensor_scalar(
                        stp[:], ct[:], ch["u"], ch["d"],
                        op0=ALU.is_ge, op1=ALU.mult)
                    state["stp"] = stp
                ops.append(op_step)

                def op_addstep():
                    acc = state["acc"]
                    tail_eng.tensor_tensor(
                        acc[:], acc[:], state["stp"][:], op=ALU.add)
                ops.append(op_addstep)

                def op_scale():
                    _, st = slots[mi]
                    acc = state["acc"]
                    tail_eng.tensor_tensor(acc[:], acc[:], st[:], op=ALU.mult)
                ops.append(op_scale)

                def op_lora():
                    wt = wpool.tile([128, fd], BF16, tag=f"w{mi}")
                    tail_eng.tensor_tensor(
                        wt[:], state["acc"][:], wl_tiles[mi][:], op=ALU.add)
                    for j, it in enumerate(range(it_lo, it_hi)):
                        w_aps[it] = wt[:, j * O_SH:(j + 1) * O_SH]
                ops.append(op_lora)
                return ops

            def dma_op(mi):
                return [lambda: emit_macro_dma(mi)]

            pending = []

            def pump(n):
                for _ in range(n):
                    if pending:
                        pending.pop(0)()

            # phase-1 macros fully upfront: m0 gates the first matmul, the
            # rest land chunk-by-chunk just ahead of block 0's consumption
            emit_macro_dma(0)
            emit_macro_dma(1)
            emit_macro_dma(2)
            emit_wl(0)
            pending += chain_ops(0)
            pump(len(pending))
            emit_wl(1)
            emit_wl(2)
            for _ in range(0, N_WARM, 2):
                pl = pp.tile([128, 2 * O_SH], F32, tag="po")
                for h in range(2):
                    nc.tensor.matmul(
                        pl[:, h * O_SH:(h + 1) * O_SH],
                        la_sb[:, 0:128], la_sb[:, 0:O_SH],
                        start=True, stop=True,
                    )
            pending += chain_ops(1) + dma_op(3) + chain_ops(2) + dma_op(4)
            pump(len(pending))
            emit_wl(3)
            emit_wl(4)
            emit_wl(5)
            pending += chain_ops(3) + dma_op(5) + chain_ops(4) + chain_ops(5)
            pump(len(pending))
            # phase-2 macros pumped through the block loop, dma one ahead
            pending += dma_op(6) + chain_ops(6) + dma_op(7) + chain_ops(7)
            pending += dma_op(8) + chain_ops(8) + dma_op(9) + chain_ops(9)
            pending += dma_op(10) + chain_ops(10)

            # ---- phase 1: M-blocked over i-tiles 0..AB_IT ----
            n_sub = AB_IT // 2
            for blk in range(NBLK):
                po_blk = []
                for sub in range(n_sub):
                    for pr_in in range(4):
                        pr = blk * 4 + pr_in
                        if sub == 0:
                            po_blk.append(pp.tile(
                                [128, 2 * O_SH], F32, tag="po",
                                name=f"po_b{blk}_{pr_in}"))
                        po = po_blk[pr_in]
                        xa = xpool.tile([128, 2, 2, 128], BF16, tag="xab",
                                        bufs=6)
                        nc.sync.dma_start(
                            xa[:],
                            xt.ap()[:, 2 * pr:2 * pr + 2,
                                    2 * sub:2 * sub + 2, :])
                        for h in range(2):
                            sub_po = po[:, h * O_SH:(h + 1) * O_SH]
                            for k in range(2):
                                nc.tensor.matmul(
                                    sub_po, xa[:, h, k, :],
                                    w_aps[2 * sub + k],
                                    start=(sub == 0 and k == 0),
                                    stop=(sub == n_sub - 1 and k == 1),
                                )
                    if sub in (1, 3):
                        pump(6)
                for pr_in in range(4):
                    pr = blk * 4 + pr_in
                    nc.scalar.copy(
                        pa[:, pr * 2 * O_SH:(pr + 1) * 2 * O_SH],
                        po_blk[pr_in][:])
                if blk < 5:
                    emit_wl(6 + blk)
            pump(len(pending))

            # ---- phase 2: i-tiles AB_IT..32, straight m-loop ----
            n_it = IT - AB_IT
            for pr in range(NPAIR):
                po = pp.tile([128, 2 * O_SH], F32, tag="po")
                for h in range(2):
                    mt = 2 * pr + h
                    xa = xpool.tile([128, n_it, 128], BF16, tag="xc", bufs=3)
                    nc.sync.dma_start(xa[:], xt.ap()[:, mt, AB_IT:, :])
                    sub_po = po[:, h * O_SH:(h + 1) * O_SH]
                    for k in range(n_it):
                        nc.tensor.matmul(
                            sub_po, xa[:, k, :], w_aps[AB_IT + k],
                            start=(k == 0), stop=(k == n_it - 1),
                        )
                pslice = pa[:, pr * 2 * O_SH:(pr + 1) * 2 * O_SH]
                ev = opool.tile([128, 2 * O_SH], F32, tag="ev")
                nc.vector.tensor_tensor(ev[:], po[:], pslice, op=ALU.add)
                dst = out.ap()[pr * 256:(pr + 1) * 256, :]
                nc.scalar.dma_start(
                    dst.rearrange("(b p) o -> p b o", b=2),
                    ev[:].rearrange("p (b o) -> p b o", b=2))

    nc.compile()
    return nc


_NC_CACHE = {}


def _get_nc():
    if "nc" not in _NC_CACHE:
        _NC_CACHE["nc"] = _build_nc()
    return _NC_CACHE["nc"]


def prepare_in_maps(x, w_codes, w_scales, lora_a, lora_b):
    """Host-side sharding + layout prep (no arithmetic beyond casts/folds)."""
    xm = np.ascontiguousarray(x.reshape(M, I))
    # xt[p, mt, t, mm] = x[mt*128+mm, t*128+p], bf16
    xtl = (
        xm.T.reshape(IT, 128, MT, 128)
        .transpose(1, 2, 0, 3)
        .astype(BF16_NP)
    )
    xtl = np.ascontiguousarray(xtl)

    la = np.ascontiguousarray(
        (LORA_SCALE * lora_a.astype(np.float64)).astype(BF16_NP)
    )

    in_maps = []
    for c in range(N_CORES):
        o_lo, o_hi = c * O_SH, (c + 1) * O_SH
        codes_t = np.ascontiguousarray(
            w_codes[o_lo:o_hi].T.astype(np.float16)
        )
        scales_t = np.ascontiguousarray(
            np.repeat(w_scales[o_lo:o_hi].T, BLK, axis=0).astype(np.float16)
        )
        lb_t = np.ascontiguousarray(lora_b[o_lo:o_hi].T.astype(BF16_NP))
        in_maps.append(
            {
                "xt": xtl,
                "codes": codes_t,
                "scales": scales_t,
                "la": la,
                "lb": lb_t,
            }
        )
    return in_maps


def run(in_maps, trace=False, retries=2):
    nc = _get_nc()
    last = None
    for attempt in range(retries + 1):
        try:
            return run_bass_kernel_spmd(
                nc, in_maps, core_ids=list(range(N_CORES)), trace=trace
            )
        except Exception as e:  # transient NRT/axon device errors
            last = e
            if attempt == retries:
                raise
            import time as _time

            _time.sleep(5)
    raise last


def kernel(x, w_codes, w_scales, lora_a, lora_b):
    in_maps = prepare_in_maps(x, w_codes, w_scales, lora_a, lora_b)
    res = run(in_maps, trace=False)
    out = np.concatenate(
        [res.results[c]["out"] for c in range(N_CORES)], axis=1
    )
    return out.reshape(B, S, O).astype(np.float32)


# revision 24
# speedup vs baseline: 1.0438x; 1.0438x over previous
"""NF4-quantized LoRA linear layer on 8 Trainium2 NeuronCores.

Computation (reference):
    w = NF4_TABLE[w_codes] * w_scales[block-expanded]        # [O, I]
    out = x @ w.T + (alpha/rank) * (x @ lora_a.T) @ lora_b.T # [B, S, O]

Strategy:
  - Tensor-parallel split of the output dim across 8 cores (O_SH = 512 each).
    Every core sees all of x; no collectives; host concatenates outputs.
  - LoRA folded into the weights per i-tile on the PE (la.T @ lb); those
    matmuls plus a dummy burst keep the PE busy from t=0 so the HAM clock
    gate is fully open (2.4 GHz) before the real matmuls start.
  - NF4 dequant: 7-term approximate chain (f16 table err 1.5e-3, end-to-end
    err ~3.5e-3 vs the 2e-2 gate): linear+step on DVE tensor_scalar, 5 relu
    ramps on ACT, 6 DVE combines, then *scales and +lora.
  - m-loop phase 1 covers i-tiles 0-11 with M BLOCKED 8-wide: each block of
    8 m-tiles keeps its 4 psum pair-tiles open across six 2-i-tile chunks,
    consuming dequant output just-in-time.  No mid-phase partial evacuation
    or re-add exists at all; each block is evacuated once to a bf16 SBUF
    partial.  Phase 2 (i-tiles 12-31) streams the remaining contraction and
    adds the partial on evacuation.
  - Dequant DMA/compute for later macros is pumped through the block loop
    so no engine FIFO head-blocks; block evacuations live on ACT only,
    phase-2 evacuation adds on DVE, output DMA on the scalar queue.
"""

import numpy as np
import ml_dtypes

import concourse.mybir as mybir
import concourse.tile as tile
from concourse import bacc
from concourse.bass_utils import run_bass_kernel_spmd

B, S, I, O, R, BLK = 4, 2048, 4096, 4096, 16, 64
M = B * S                      # 8192 token rows
N_CORES = 8
O_SH = O // N_CORES            # 512 output cols per core
IT = I // 128                  # 32 contraction tiles
MT = M // 128                  # 64 row tiles
NPAIR = MT // 2                # 32 psum pair-tiles per phase
LORA_SCALE = 2.0               # alpha / rank

# dequant macros: six 2-i-tile chunks feed phase 1 just-in-time, then five
# 4-i-tile macros for phase 2
MACROS = [2, 2, 2, 2, 2, 2, 4, 4, 4, 4, 4]
AB_IT = 12                     # i-tiles covered by phase 1 (macros 0-5)
GP_TAIL_FROM = 6               # macros >= this run the chain tail on GPSIMD
N_WARM = 14                    # dummy warm-up matmuls
NBLK = 8                       # m-blocks in phase 1 (8 m-tiles each)

# NF4 chains: t(c) ~= a + b*c + sum_j g_j*relu(c - v_j) + d*[c>=13.5]
# CH7: 5 ramps, f16 table err 1.45e-3 -- used for phase-2 macros.
# CH5: 3 ramps, f16 table err 9.3e-3 -- used for the startup-critical
# phase-1 macros (12/32 of the contraction; total output err ~6.5e-3
# vs the 2e-2 gate).
CH7 = dict(
    a=-0.9999999999955771, b=0.3037613463764206,
    d=-0.11607743835394424, u=13.5,
    ramps=[
        (0.17424857616421482, 12.890314243043882),
        (-0.0147269920683398, 6.461280539039212),
        (-0.17365163565386407, 1.2363687528522225),
        (0.020825906737021872, 10.455589664724952),
        (-0.033414218483025136, 3.450174298600788),
    ])
CH5 = dict(
    a=-1.0000000000315237, b=0.3038071989637578,
    d=-0.11670333147042945, u=13.5,
    ramps=[
        (0.18864440149390185, 12.775812349363168),
        (-0.041756800433337744, 3.7380006069052687),
        (-0.17365163624795468, 1.2386672442106303),
    ])
N_BIAS = len(CH7["ramps"]) + len(CH5["ramps"])

F16 = mybir.dt.float16
BF16 = mybir.dt.bfloat16
F32 = mybir.dt.float32
ALU = mybir.AluOpType
ACTF = mybir.ActivationFunctionType

BF16_NP = ml_dtypes.bfloat16


def _macro_ranges():
    out, lo = [], 0
    for n in MACROS:
        out.append((lo, lo + n))
        lo += n
    return out


def _build_nc():
    nc = bacc.Bacc("TRN2", target_bir_lowering=False, debug=False,
                   num_devices=N_CORES)

    xt = nc.dram_tensor("xt", [128, MT, IT, 128], BF16, kind="ExternalInput")
    xab = nc.dram_tensor("xab", [128, NPAIR, AB_IT // 2, 512], BF16,
                         kind="ExternalInput")
    codes = nc.dram_tensor("codes", [I, O_SH], F16, kind="ExternalInput")
    scales = nc.dram_tensor("scales", [I, O_SH], F16, kind="ExternalInput")
    la = nc.dram_tensor("la", [R, I], BF16, kind="ExternalInput")
    lb = nc.dram_tensor("lb", [R, O_SH], BF16, kind="ExternalInput")
    out = nc.dram_tensor("out", [M, O_SH], F32, kind="ExternalOutput")

    codes_r = codes.ap().rearrange("(t p) o -> p t o", p=128)
    scales_r = scales.ap().rearrange("(t p) o -> p t o", p=128)
    mranges = _macro_ranges()

    with tile.TileContext(nc) as tc:
        with (
            tc.tile_pool(name="wpool", bufs=1) as wpool,
            tc.tile_pool(name="wlab", bufs=6) as wlab,
            tc.tile_pool(name="wlc", bufs=2) as wlc,
            tc.tile_pool(name="dqio", bufs=2) as dqio,
            tc.tile_pool(name="dq", bufs=2) as dq,
            tc.tile_pool(name="xpool", bufs=3) as xpool,
            tc.tile_pool(name="cpool", bufs=1) as cpool,
            tc.tile_pool(name="opool", bufs=3) as opool,
            tc.tile_pool(name="ps", bufs=4, space="PSUM") as pp,
        ):
            # ---- constants ----
            la_sb = cpool.tile([R, I], BF16, tag="la")
            nc.gpsimd.dma_start(la_sb[:], la.ap())
            lb_sb = cpool.tile([R, O_SH], BF16, tag="lb")
            nc.gpsimd.dma_start(lb_sb[:], lb.ap())
            biases = cpool.tile([128, N_BIAS], F32, tag="bias")
            for j, (g, v) in enumerate(CH7["ramps"] + CH5["ramps"]):
                nc.vector.memset(biases[:, j:j + 1], -abs(g) * v)
            # SBUF bf16 partial accumulator [128, MT*512]
            pa = cpool.tile([128, MT * O_SH], BF16, tag="pa")

            # ---- wl (lora fold) + dummy warm-up on the PE ----
            wl_tiles = {}

            def emit_wl(mi):
                it_lo, it_hi = mranges[mi]
                nt = it_hi - it_lo
                pool = wlab if mi < 6 else wlc
                wl = pool.tile([128, nt * O_SH], F16, tag="wl")
                j = 0
                cnt = 0
                while j < nt:
                    k = min(2, nt - j)
                    pl = pp.tile([128, 2 * O_SH], F32, tag="po")
                    for h in range(k):
                        it = it_lo + j + h
                        nc.tensor.matmul(
                            pl[:, h * O_SH:(h + 1) * O_SH],
                            la_sb[:, it * 128:(it + 1) * 128], lb_sb[:],
                            start=True, stop=True,
                        )
                    dst = wl[:, j * O_SH:(j + k) * O_SH]
                    if cnt % 2 == 0:
                        nc.scalar.copy(dst, pl[:, :k * O_SH])
                    else:
                        nc.vector.tensor_copy(dst, pl[:, :k * O_SH])
                    cnt += 1
                    j += k
                wl_tiles[mi] = wl

            # ---- dequant ----
            w_aps = {}
            slots = {}

            def emit_macro_dma(mi):
                it_lo, it_hi = mranges[mi]
                nt = it_hi - it_lo
                fd = nt * O_SH
                ct = dqio.tile([128, fd], F16, tag="ct")
                nc.gpsimd.dma_start(
                    ct[:].rearrange("p (t o) -> p t o", t=nt),
                    codes_r[:, it_lo:it_hi, :],
                )
                st = dqio.tile([128, fd], F16, tag="st")
                nc.gpsimd.dma_start(
                    st[:].rearrange("p (t o) -> p t o", t=nt),
                    scales_r[:, it_lo:it_hi, :],
                )
                slots[mi] = (ct, st)

            def chain_ops(mi):
                it_lo, it_hi = mranges[mi]
                nt = it_hi - it_lo
                fd = nt * O_SH
                tail_eng = nc.gpsimd if mi >= GP_TAIL_FROM else nc.vector
                ch = CH5 if mi < 6 else CH7
                boff = len(CH7["ramps"]) if mi < 6 else 0
                state = {}
                ops = []

                def op_lin():
                    ct, _ = slots[mi]
                    acc = dq.tile([128, fd], F16, tag="acc")
                    nc.vector.tensor_scalar(
                        acc[:], ct[:], ch["b"], ch["a"],
                        op0=ALU.mult, op1=ALU.add)
                    state["acc"] = acc
                ops.append(op_lin)
                for j, (g, v) in enumerate(ch["ramps"]):
                    def op_ramp(j=j, g=g):
                        ct, _ = slots[mi]
                        r = dq.tile([128, fd], F16, tag="rmp")
                        nc.scalar.activation(
                            r[:], ct[:], ACTF.Relu,
                            bias=biases[:, boff + j:boff + j + 1],
                            scale=abs(g))
                        state["r"] = r
                    ops.append(op_ramp)

                    def op_comb(g=g):
                        acc = state["acc"]
                        nc.vector.tensor_tensor(
                            acc[:], acc[:], state["r"][:],
                            op=ALU.add if g > 0 else ALU.subtract)
                    ops.append(op_comb)

                def op_step():
                    ct, _ = slots[mi]
                    stp = dq.tile([128, fd], F16, tag="rmp")
                    nc.vector.tensor_scalar(
                        stp[:], ct[:], ch["u"], ch["d"],
                        op0=ALU.is_ge, op1=ALU.mult)
                    state["stp"] = stp
                ops.append(op_step)

                def op_addstep():
                    acc = state["acc"]
                    tail_eng.tensor_tensor(
                        acc[:], acc[:], state["stp"][:], op=ALU.add)
                ops.append(op_addstep)

                def op_scale():
                    _, st = slots[mi]
                    acc = state["acc"]
                    tail_eng.tensor_tensor(acc[:], acc[:], st[:], op=ALU.mult)
                ops.append(op_scale)

                def op_lora():
                    wt = wpool.tile([128, fd], BF16, tag=f"w{mi}")
                    tail_eng.tensor_tensor(
                        wt[:], state["acc"][:], wl_tiles[mi][:], op=ALU.add)
                    for j, it in enumerate(range(it_lo, it_hi)):
                        w_aps[it] = wt[:, j * O_SH:(j + 1) * O_SH]
                ops.append(op_lora)
                return ops

            def dma_op(mi):
                return [lambda: emit_macro_dma(mi)]

            pending = []

            def pump(n):
                for _ in range(n):
                    if pending:
                        pending.pop(0)()

            # phase-1 macros fully upfront: m0 gates the first matmul, the
            # rest land chunk-by-chunk just ahead of block 0's consumption
            emit_macro_dma(0)
            emit_macro_dma(1)
            emit_macro_dma(2)
            emit_wl(0)
            pending += chain_ops(0)
            pump(len(pending))
            emit_wl(1)
            emit_wl(2)
            for _ in range(0, N_WARM, 2):
                pl = pp.tile([128, 2 * O_SH], F32, tag="po")
                for h in range(2):
                    nc.tensor.matmul(
                        pl[:, h * O_SH:(h + 1) * O_SH],
                        la_sb[:, 0:128], la_sb[:, 0:O_SH],
                        start=True, stop=True,
                    )
            pending += chain_ops(1) + dma_op(3) + chain_ops(2) + dma_op(4)
            pump(len(pending))
            emit_wl(3)
            emit_wl(4)
            emit_wl(5)
            pending += chain_ops(3) + dma_op(5) + chain_ops(4) + chain_ops(5)
            pump(len(pending))
            # phase-2 macros pumped through the block loop, dma one ahead
            pending += dma_op(6) + chain_ops(6) + dma_op(7) + chain_ops(7)
            pending += dma_op(8) + chain_ops(8) + dma_op(9) + chain_ops(9)
            pending += dma_op(10) + chain_ops(10)

            # ---- phase 1: M-blocked over i-tiles 0..AB_IT ----
            n_sub = AB_IT // 2
            for blk in range(NBLK):
                po_blk = []
                for sub in range(n_sub):
                    for pr_in in range(4):
                        pr = blk * 4 + pr_in
                        if sub == 0:
                            po_blk.append(pp.tile(
                                [128, 2 * O_SH], F32, tag="po",
                                name=f"po_b{blk}_{pr_in}"))
                        po = po_blk[pr_in]
                        xa = xpool.tile([128, 512], BF16, tag="xab",
                                        bufs=6)
                        nc.sync.dma_start(xa[:], xab.ap()[:, pr, sub, :])
                        for h in range(2):
                            sub_po = po[:, h * O_SH:(h + 1) * O_SH]
                            for k in range(2):
                                off = (h * 2 + k) * 128
                                nc.tensor.matmul(
                                    sub_po, xa[:, off:off + 128],
                                    w_aps[2 * sub + k],
                                    start=(sub == 0 and k == 0),
                                    stop=(sub == n_sub - 1 and k == 1),
                                )
                    if sub in (1, 3):
                        pump(6)
                for pr_in in range(4):
                    pr = blk * 4 + pr_in
                    nc.scalar.copy(
                        pa[:, pr * 2 * O_SH:(pr + 1) * 2 * O_SH],
                        po_blk[pr_in][:])
                if blk < 5:
                    emit_wl(6 + blk)
            pump(len(pending))

            # ---- phase 2: i-tiles AB_IT..32, straight m-loop ----
            n_it = IT - AB_IT
            for pr in range(NPAIR):
                po = pp.tile([128, 2 * O_SH], F32, tag="po")
                for h in range(2):
                    mt = 2 * pr + h
                    xa = xpool.tile([128, n_it, 128], BF16, tag="xc", bufs=3)
                    nc.sync.dma_start(xa[:], xt.ap()[:, mt, AB_IT:, :])
                    sub_po = po[:, h * O_SH:(h + 1) * O_SH]
                    for k in range(n_it):
                        nc.tensor.matmul(
                            sub_po, xa[:, k, :], w_aps[AB_IT + k],
                            start=(k == 0), stop=(k == n_it - 1),
                        )
                pslice = pa[:, pr * 2 * O_SH:(pr + 1) * 2 * O_SH]
                ev = opool.tile([128, 2 * O_SH], F32, tag="ev")
                nc.vector.tensor_tensor(ev[:], po[:], pslice, op=ALU.add)
                dst = out.ap()[pr * 256:(pr + 1) * 256, :]
                nc.scalar.dma_start(
                    dst.rearrange("(b p) o -> p b o", b=2),
                    ev[:].rearrange("p (b o) -> p b o", b=2))

    nc.compile()
    return nc


_NC_CACHE = {}


def _get_nc():
    if "nc" not in _NC_CACHE:
        _NC_CACHE["nc"] = _build_nc()
    return _NC_CACHE["nc"]


def prepare_in_maps(x, w_codes, w_scales, lora_a, lora_b):
    """Host-side sharding + layout prep (no arithmetic beyond casts/folds)."""
    xm = np.ascontiguousarray(x.reshape(M, I))
    # xt[p, mt, t, mm] = x[mt*128+mm, t*128+p], bf16
    xtl = (
        xm.T.reshape(IT, 128, MT, 128)
        .transpose(1, 2, 0, 3)
        .astype(BF16_NP)
    )
    xtl = np.ascontiguousarray(xtl)
    # block-major layout for phase 1: xab[p, pr, sub, (h k mm)]
    nsub = AB_IT // 2
    xabl = (
        xtl[:, :, :AB_IT, :]
        .reshape(128, NPAIR, 2, nsub, 2, 128)
        .transpose(0, 1, 3, 2, 4, 5)
        .reshape(128, NPAIR, nsub, 512)
    )
    xabl = np.ascontiguousarray(xabl)

    la = np.ascontiguousarray(
        (LORA_SCALE * lora_a.astype(np.float64)).astype(BF16_NP)
    )

    in_maps = []
    for c in range(N_CORES):
        o_lo, o_hi = c * O_SH, (c + 1) * O_SH
        codes_t = np.ascontiguousarray(
            w_codes[o_lo:o_hi].T.astype(np.float16)
        )
        scales_t = np.ascontiguousarray(
            np.repeat(w_scales[o_lo:o_hi].T, BLK, axis=0).astype(np.float16)
        )
        lb_t = np.ascontiguousarray(lora_b[o_lo:o_hi].T.astype(BF16_NP))
        in_maps.append(
            {
                "xt": xtl,
                "xab": xabl,
                "codes": codes_t,
                "scales": scales_t,
                "la": la,
                "lb": lb_t,
            }
        )
    return in_maps


def run(in_maps, trace=False, retries=2):
    nc = _get_nc()
    last = None
    for attempt in range(retries + 1):
        try:
            return run_bass_kernel_spmd(
                nc, in_maps, core_ids=list(range(N_CORES)), trace=trace
            )
        except Exception as e:  # transient NRT/axon device errors
            last = e
            if attempt == retries:
                raise
            import time as _time

            _time.sleep(5)
    raise last


def kernel(x, w_codes, w_scales, lora_a, lora_b):
    in_maps = prepare_in_maps(x, w_codes, w_scales, lora_a, lora_b)
    res = run(in_maps, trace=False)
    out = np.concatenate(
        [res.results[c]["out"] for c in range(N_CORES)], axis=1
    )
    return out.reshape(B, S, O).astype(np.float32)


# revision 25
# speedup vs baseline: 1.0491x; 1.0050x over previous
"""NF4-quantized LoRA linear layer on 8 Trainium2 NeuronCores.

Computation (reference):
    w = NF4_TABLE[w_codes] * w_scales[block-expanded]        # [O, I]
    out = x @ w.T + (alpha/rank) * (x @ lora_a.T) @ lora_b.T # [B, S, O]

Strategy:
  - Tensor-parallel split of the output dim across 8 cores (O_SH = 512 each).
    Every core sees all of x; no collectives; host concatenates outputs.
  - LoRA folded into the weights per i-tile on the PE (la.T @ lb); those
    matmuls plus a dummy burst keep the PE busy from t=0 so the HAM clock
    gate is fully open (2.4 GHz) before the real matmuls start.
  - NF4 dequant: 7-term approximate chain (f16 table err 1.5e-3, end-to-end
    err ~3.5e-3 vs the 2e-2 gate): linear+step on DVE tensor_scalar, 5 relu
    ramps on ACT, 6 DVE combines, then *scales and +lora.
  - m-loop phase 1 covers i-tiles 0-11 with M BLOCKED 8-wide: each block of
    8 m-tiles keeps its 4 psum pair-tiles open across six 2-i-tile chunks,
    consuming dequant output just-in-time.  No mid-phase partial evacuation
    or re-add exists at all; each block is evacuated once to a bf16 SBUF
    partial.  Phase 2 (i-tiles 12-31) streams the remaining contraction and
    adds the partial on evacuation.
  - Dequant DMA/compute for later macros is pumped through the block loop
    so no engine FIFO head-blocks; block evacuations live on ACT only,
    phase-2 evacuation adds on DVE, output DMA on the scalar queue.
"""

import numpy as np
import ml_dtypes

import concourse.mybir as mybir
import concourse.tile as tile
from concourse import bacc
from concourse.bass_utils import run_bass_kernel_spmd

B, S, I, O, R, BLK = 4, 2048, 4096, 4096, 16, 64
M = B * S                      # 8192 token rows
N_CORES = 8
O_SH = O // N_CORES            # 512 output cols per core
IT = I // 128                  # 32 contraction tiles
MT = M // 128                  # 64 row tiles
NPAIR = MT // 2                # 32 psum pair-tiles per phase
LORA_SCALE = 2.0               # alpha / rank

# dequant macros: six 2-i-tile chunks feed phase 1 just-in-time, then five
# 4-i-tile macros for phase 2
MACROS = [2, 2, 2, 2, 2, 4, 4, 4, 4, 4, 2]
AB_IT = 10                     # i-tiles covered by phase 1 (macros 0-4)
GP_TAIL_FROM = 5               # macros >= this run the chain tail on GPSIMD
N_WARM = 14                    # dummy warm-up matmuls
NBLK = 8                       # m-blocks in phase 1 (8 m-tiles each)

# NF4 chains: t(c) ~= a + b*c + sum_j g_j*relu(c - v_j) + d*[c>=13.5]
# CH7: 5 ramps, f16 table err 1.45e-3 -- used for phase-2 macros.
# CH5: 3 ramps, f16 table err 9.3e-3 -- used for the startup-critical
# phase-1 macros (12/32 of the contraction; total output err ~6.5e-3
# vs the 2e-2 gate).
CH7 = dict(
    a=-0.9999999999955771, b=0.3037613463764206,
    d=-0.11607743835394424, u=13.5,
    ramps=[
        (0.17424857616421482, 12.890314243043882),
        (-0.0147269920683398, 6.461280539039212),
        (-0.17365163565386407, 1.2363687528522225),
        (0.020825906737021872, 10.455589664724952),
        (-0.033414218483025136, 3.450174298600788),
    ])
CH5 = dict(
    a=-1.0000000000315237, b=0.3038071989637578,
    d=-0.11670333147042945, u=13.5,
    ramps=[
        (0.18864440149390185, 12.775812349363168),
        (-0.041756800433337744, 3.7380006069052687),
        (-0.17365163624795468, 1.2386672442106303),
    ])
N_BIAS = len(CH7["ramps"]) + len(CH5["ramps"])

F16 = mybir.dt.float16
BF16 = mybir.dt.bfloat16
F32 = mybir.dt.float32
ALU = mybir.AluOpType
ACTF = mybir.ActivationFunctionType

BF16_NP = ml_dtypes.bfloat16


def _macro_ranges():
    out, lo = [], 0
    for n in MACROS:
        out.append((lo, lo + n))
        lo += n
    return out


def _build_nc():
    nc = bacc.Bacc("TRN2", target_bir_lowering=False, debug=False,
                   num_devices=N_CORES)

    xt = nc.dram_tensor("xt", [128, MT, IT, 128], BF16, kind="ExternalInput")
    xab = nc.dram_tensor("xab", [128, NPAIR, AB_IT // 2, 512], BF16,
                         kind="ExternalInput")
    codes = nc.dram_tensor("codes", [I, O_SH], F16, kind="ExternalInput")
    scales = nc.dram_tensor("scales", [I, O_SH], F16, kind="ExternalInput")
    la = nc.dram_tensor("la", [R, I], BF16, kind="ExternalInput")
    lb = nc.dram_tensor("lb", [R, O_SH], BF16, kind="ExternalInput")
    out = nc.dram_tensor("out", [M, O_SH], F32, kind="ExternalOutput")

    codes_r = codes.ap().rearrange("(t p) o -> p t o", p=128)
    scales_r = scales.ap().rearrange("(t p) o -> p t o", p=128)
    mranges = _macro_ranges()

    with tile.TileContext(nc) as tc:
        with (
            tc.tile_pool(name="wpool", bufs=1) as wpool,
            tc.tile_pool(name="wlab", bufs=5) as wlab,
            tc.tile_pool(name="wlc", bufs=2) as wlc,
            tc.tile_pool(name="dqio", bufs=2) as dqio,
            tc.tile_pool(name="dq", bufs=2) as dq,
            tc.tile_pool(name="xpool", bufs=3) as xpool,
            tc.tile_pool(name="cpool", bufs=1) as cpool,
            tc.tile_pool(name="opool", bufs=3) as opool,
            tc.tile_pool(name="ps", bufs=4, space="PSUM") as pp,
        ):
            # ---- constants ----
            la_sb = cpool.tile([R, I], BF16, tag="la")
            nc.gpsimd.dma_start(la_sb[:], la.ap())
            lb_sb = cpool.tile([R, O_SH], BF16, tag="lb")
            nc.gpsimd.dma_start(lb_sb[:], lb.ap())
            biases = cpool.tile([128, N_BIAS], F32, tag="bias")
            for j, (g, v) in enumerate(CH7["ramps"] + CH5["ramps"]):
                nc.vector.memset(biases[:, j:j + 1], -abs(g) * v)
            # SBUF bf16 partial accumulator [128, MT*512]
            pa = cpool.tile([128, MT * O_SH], BF16, tag="pa")

            # ---- wl (lora fold) + dummy warm-up on the PE ----
            wl_tiles = {}

            def emit_wl(mi):
                it_lo, it_hi = mranges[mi]
                nt = it_hi - it_lo
                pool = wlab if mi < 5 else wlc
                wl = pool.tile([128, nt * O_SH], F16, tag="wl")
                j = 0
                cnt = 0
                while j < nt:
                    k = min(2, nt - j)
                    pl = pp.tile([128, 2 * O_SH], F32, tag="po")
                    for h in range(k):
                        it = it_lo + j + h
                        nc.tensor.matmul(
                            pl[:, h * O_SH:(h + 1) * O_SH],
                            la_sb[:, it * 128:(it + 1) * 128], lb_sb[:],
                            start=True, stop=True,
                        )
                    dst = wl[:, j * O_SH:(j + k) * O_SH]
                    if cnt % 2 == 0:
                        nc.scalar.copy(dst, pl[:, :k * O_SH])
                    else:
                        nc.vector.tensor_copy(dst, pl[:, :k * O_SH])
                    cnt += 1
                    j += k
                wl_tiles[mi] = wl

            # ---- dequant ----
            w_aps = {}
            slots = {}

            def emit_macro_dma(mi):
                it_lo, it_hi = mranges[mi]
                nt = it_hi - it_lo
                fd = nt * O_SH
                ct = dqio.tile([128, fd], F16, tag="ct")
                nc.gpsimd.dma_start(
                    ct[:].rearrange("p (t o) -> p t o", t=nt),
                    codes_r[:, it_lo:it_hi, :],
                )
                st = dqio.tile([128, fd], F16, tag="st")
                nc.gpsimd.dma_start(
                    st[:].rearrange("p (t o) -> p t o", t=nt),
                    scales_r[:, it_lo:it_hi, :],
                )
                slots[mi] = (ct, st)

            def chain_ops(mi):
                it_lo, it_hi = mranges[mi]
                nt = it_hi - it_lo
                fd = nt * O_SH
                tail_eng = nc.gpsimd if mi >= GP_TAIL_FROM else nc.vector
                ch = CH5 if mi < 5 else CH7
                boff = len(CH7["ramps"]) if mi < 5 else 0
                state = {}
                ops = []

                def op_lin():
                    ct, _ = slots[mi]
                    acc = dq.tile([128, fd], F16, tag="acc")
                    nc.vector.tensor_scalar(
                        acc[:], ct[:], ch["b"], ch["a"],
                        op0=ALU.mult, op1=ALU.add)
                    state["acc"] = acc
                ops.append(op_lin)
                for j, (g, v) in enumerate(ch["ramps"]):
                    def op_ramp(j=j, g=g):
                        ct, _ = slots[mi]
                        r = dq.tile([128, fd], F16, tag="rmp")
                        nc.scalar.activation(
                            r[:], ct[:], ACTF.Relu,
                            bias=biases[:, boff + j:boff + j + 1],
                            scale=abs(g))
                        state["r"] = r
                    ops.append(op_ramp)

                    def op_comb(g=g):
                        acc = state["acc"]
                        nc.vector.tensor_tensor(
                            acc[:], acc[:], state["r"][:],
                            op=ALU.add if g > 0 else ALU.subtract)
                    ops.append(op_comb)

                def op_step():
                    ct, _ = slots[mi]
                    stp = dq.tile([128, fd], F16, tag="rmp")
                    nc.vector.tensor_scalar(
                        stp[:], ct[:], ch["u"], ch["d"],
                        op0=ALU.is_ge, op1=ALU.mult)
                    state["stp"] = stp
                ops.append(op_step)

                def op_addstep():
                    acc = state["acc"]
                    tail_eng.tensor_tensor(
                        acc[:], acc[:], state["stp"][:], op=ALU.add)
                ops.append(op_addstep)

                def op_scale():
                    _, st = slots[mi]
                    acc = state["acc"]
                    tail_eng.tensor_tensor(acc[:], acc[:], st[:], op=ALU.mult)
                ops.append(op_scale)

                def op_lora():
                    wt = wpool.tile([128, fd], BF16, tag=f"w{mi}")
                    tail_eng.tensor_tensor(
                        wt[:], state["acc"][:], wl_tiles[mi][:], op=ALU.add)
                    for j, it in enumerate(range(it_lo, it_hi)):
                        w_aps[it] = wt[:, j * O_SH:(j + 1) * O_SH]
                ops.append(op_lora)
                return ops

            def dma_op(mi):
                return [lambda: emit_macro_dma(mi)]

            pending = []

            def pump(n):
                for _ in range(n):
                    if pending:
                        pending.pop(0)()

            # phase-1 macros fully upfront: m0 gates the first matmul, the
            # rest land chunk-by-chunk just ahead of block 0's consumption
            emit_macro_dma(0)
            emit_macro_dma(1)
            emit_macro_dma(2)
            emit_wl(0)
            pending += chain_ops(0)
            pump(len(pending))
            emit_wl(1)
            emit_wl(2)
            for _ in range(0, N_WARM, 2):
                pl = pp.tile([128, 2 * O_SH], F32, tag="po")
                for h in range(2):
                    nc.tensor.matmul(
                        pl[:, h * O_SH:(h + 1) * O_SH],
                        la_sb[:, 0:128], la_sb[:, 0:O_SH],
                        start=True, stop=True,
                    )
            pending += chain_ops(1) + dma_op(3) + chain_ops(2) + dma_op(4)
            pump(len(pending))
            emit_wl(3)
            emit_wl(4)
            pending += chain_ops(3) + dma_op(5) + chain_ops(4)
            pump(len(pending))
            # phase-2 macros pumped through the block loop, dma one ahead
            pending += chain_ops(5) + dma_op(6) + chain_ops(6)
            pending += dma_op(7) + chain_ops(7) + dma_op(8) + chain_ops(8)
            pending += dma_op(9) + chain_ops(9) + dma_op(10) + chain_ops(10)

            # ---- phase 1: M-blocked over i-tiles 0..AB_IT ----
            n_sub = AB_IT // 2
            for blk in range(NBLK):
                po_blk = []
                for sub in range(n_sub):
                    for pr_in in range(4):
                        pr = blk * 4 + pr_in
                        if sub == 0:
                            po_blk.append(pp.tile(
                                [128, 2 * O_SH], F32, tag="po",
                                name=f"po_b{blk}_{pr_in}"))
                        po = po_blk[pr_in]
                        xa = xpool.tile([128, 512], BF16, tag="xab",
                                        bufs=8)
                        nc.sync.dma_start(xa[:], xab.ap()[:, pr, sub, :])
                        for h in range(2):
                            sub_po = po[:, h * O_SH:(h + 1) * O_SH]
                            for k in range(2):
                                off = (h * 2 + k) * 128
                                nc.tensor.matmul(
                                    sub_po, xa[:, off:off + 128],
                                    w_aps[2 * sub + k],
                                    start=(sub == 0 and k == 0),
                                    stop=(sub == n_sub - 1 and k == 1),
                                )
                    if sub in (1, 3):
                        pump(6)
                for pr_in in range(4):
                    pr = blk * 4 + pr_in
                    nc.scalar.copy(
                        pa[:, pr * 2 * O_SH:(pr + 1) * 2 * O_SH],
                        po_blk[pr_in][:])
                if blk < 6:
                    emit_wl(5 + blk)
            pump(len(pending))

            # ---- phase 2: i-tiles AB_IT..32, straight m-loop ----
            n_it = IT - AB_IT
            for pr in range(NPAIR):
                po = pp.tile([128, 2 * O_SH], F32, tag="po")
                for h in range(2):
                    mt = 2 * pr + h
                    xa = xpool.tile([128, n_it, 128], BF16, tag="xc", bufs=3)
                    nc.sync.dma_start(xa[:], xt.ap()[:, mt, AB_IT:, :])
                    sub_po = po[:, h * O_SH:(h + 1) * O_SH]
                    for k in range(n_it):
                        nc.tensor.matmul(
                            sub_po, xa[:, k, :], w_aps[AB_IT + k],
                            start=(k == 0), stop=(k == n_it - 1),
                        )
                pslice = pa[:, pr * 2 * O_SH:(pr + 1) * 2 * O_SH]
                ev = opool.tile([128, 2 * O_SH], F32, tag="ev")
                nc.vector.tensor_tensor(ev[:], po[:], pslice, op=ALU.add)
                dst = out.ap()[pr * 256:(pr + 1) * 256, :]
                nc.scalar.dma_start(
                    dst.rearrange("(b p) o -> p b o", b=2),
                    ev[:].rearrange("p (b o) -> p b o", b=2))

    nc.compile()
    return nc


_NC_CACHE = {}


def _get_nc():
    if "nc" not in _NC_CACHE:
        _NC_CACHE["nc"] = _build_nc()
    return _NC_CACHE["nc"]


def prepare_in_maps(x, w_codes, w_scales, lora_a, lora_b):
    """Host-side sharding + layout prep (no arithmetic beyond casts/folds)."""
    xm = np.ascontiguousarray(x.reshape(M, I))
    # xt[p, mt, t, mm] = x[mt*128+mm, t*128+p], bf16
    xtl = (
        xm.T.reshape(IT, 128, MT, 128)
        .transpose(1, 2, 0, 3)
        .astype(BF16_NP)
    )
    xtl = np.ascontiguousarray(xtl)
    # block-major layout for phase 1: xab[p, pr, sub, (h k mm)]
    nsub = AB_IT // 2
    xabl = (
        xtl[:, :, :AB_IT, :]
        .reshape(128, NPAIR, 2, nsub, 2, 128)
        .transpose(0, 1, 3, 2, 4, 5)
        .reshape(128, NPAIR, nsub, 512)
    )
    xabl = np.ascontiguousarray(xabl)

    la = np.ascontiguousarray(
        (LORA_SCALE * lora_a.astype(np.float64)).astype(BF16_NP)
    )

    in_maps = []
    for c in range(N_CORES):
        o_lo, o_hi = c * O_SH, (c + 1) * O_SH
        codes_t = np.ascontiguousarray(
            w_codes[o_lo:o_hi].T.astype(np.float16)
        )
        scales_t = np.ascontiguousarray(
            np.repeat(w_scales[o_lo:o_hi].T, BLK, axis=0).astype(np.float16)
        )
        lb_t = np.ascontiguousarray(lora_b[o_lo:o_hi].T.astype(BF16_NP))
        in_maps.append(
            {
                "xt": xtl,
                "xab": xabl,
                "codes": codes_t,
                "scales": scales_t,
                "la": la,
                "lb": lb_t,
            }
        )
    return in_maps


def run(in_maps, trace=False, retries=2):
    nc = _get_nc()
    last = None
    for attempt in range(retries + 1):
        try:
            return run_bass_kernel_spmd(
                nc, in_maps, core_ids=list(range(N_CORES)), trace=trace
            )
        except Exception as e:  # transient NRT/axon device errors
            last = e
            if attempt == retries:
                raise
            import time as _time

            _time.sleep(5)
    raise last


def kernel(x, w_codes, w_scales, lora_a, lora_b):
    in_maps = prepare_in_maps(x, w_codes, w_scales, lora_a, lora_b)
    res = run(in_maps, trace=False)
    out = np.concatenate(
        [res.results[c]["out"] for c in range(N_CORES)], axis=1
    )
    return out.reshape(B, S, O).astype(np.float32)


# revision 26
# speedup vs baseline: 1.0520x; 1.0028x over previous
"""NF4-quantized LoRA linear layer on 8 Trainium2 NeuronCores.

Computation (reference):
    w = NF4_TABLE[w_codes] * w_scales[block-expanded]        # [O, I]
    out = x @ w.T + (alpha/rank) * (x @ lora_a.T) @ lora_b.T # [B, S, O]

Strategy:
  - Tensor-parallel split of the output dim across 8 cores (O_SH = 512 each).
    Every core sees all of x; no collectives; host concatenates outputs.
  - LoRA folded into the weights per i-tile on the PE (la.T @ lb); those
    matmuls plus a dummy burst keep the PE busy from t=0 so the HAM clock
    gate is fully open (2.4 GHz) before the real matmuls start.
  - NF4 dequant: 7-term approximate chain (f16 table err 1.5e-3, end-to-end
    err ~3.5e-3 vs the 2e-2 gate): linear+step on DVE tensor_scalar, 5 relu
    ramps on ACT, 6 DVE combines, then *scales and +lora.
  - m-loop phase 1 covers i-tiles 0-11 with M BLOCKED 8-wide: each block of
    8 m-tiles keeps its 4 psum pair-tiles open across six 2-i-tile chunks,
    consuming dequant output just-in-time.  No mid-phase partial evacuation
    or re-add exists at all; each block is evacuated once to a bf16 SBUF
    partial.  Phase 2 (i-tiles 12-31) streams the remaining contraction and
    adds the partial on evacuation.
  - Dequant DMA/compute for later macros is pumped through the block loop
    so no engine FIFO head-blocks; block evacuations live on ACT only,
    phase-2 evacuation adds on DVE, output DMA on the scalar queue.
"""

import numpy as np
import ml_dtypes

import concourse.mybir as mybir
import concourse.tile as tile
from concourse import bacc
from concourse.bass_utils import run_bass_kernel_spmd

B, S, I, O, R, BLK = 4, 2048, 4096, 4096, 16, 64
M = B * S                      # 8192 token rows
N_CORES = 8
O_SH = O // N_CORES            # 512 output cols per core
IT = I // 128                  # 32 contraction tiles
MT = M // 128                  # 64 row tiles
NPAIR = MT // 2                # 32 psum pair-tiles per phase
LORA_SCALE = 2.0               # alpha / rank

# dequant macros: six 2-i-tile chunks feed phase 1 just-in-time, then five
# 4-i-tile macros for phase 2
MACROS = [2, 2, 2, 2, 2, 4, 4, 4, 4, 4, 2]
AB_IT = 10                     # i-tiles covered by phase 1 (macros 0-4)
GP_TAIL_FROM = 5               # macros >= this run the chain tail on GPSIMD
N_WARM = 20                    # dummy warm-up matmuls
NBLK = 8                       # m-blocks in phase 1 (8 m-tiles each)

# NF4 chains: t(c) ~= a + b*c + sum_j g_j*relu(c - v_j) + d*[c>=13.5]
# CH7: 5 ramps, f16 table err 1.45e-3 -- used for phase-2 macros.
# CH5: 3 ramps, f16 table err 9.3e-3 -- used for the startup-critical
# phase-1 macros (12/32 of the contraction; total output err ~6.5e-3
# vs the 2e-2 gate).
CH7 = dict(
    a=-0.9999999999955771, b=0.3037613463764206,
    d=-0.11607743835394424, u=13.5,
    ramps=[
        (0.17424857616421482, 12.890314243043882),
        (-0.0147269920683398, 6.461280539039212),
        (-0.17365163565386407, 1.2363687528522225),
        (0.020825906737021872, 10.455589664724952),
        (-0.033414218483025136, 3.450174298600788),
    ])
CH5 = dict(
    a=-1.0000000000315237, b=0.3038071989637578,
    d=-0.11670333147042945, u=13.5,
    ramps=[
        (0.18864440149390185, 12.775812349363168),
        (-0.041756800433337744, 3.7380006069052687),
        (-0.17365163624795468, 1.2386672442106303),
    ])
N_BIAS = len(CH7["ramps"]) + len(CH5["ramps"])

F16 = mybir.dt.float16
BF16 = mybir.dt.bfloat16
F32 = mybir.dt.float32
ALU = mybir.AluOpType
ACTF = mybir.ActivationFunctionType

BF16_NP = ml_dtypes.bfloat16


def _macro_ranges():
    out, lo = [], 0
    for n in MACROS:
        out.append((lo, lo + n))
        lo += n
    return out


def _build_nc():
    nc = bacc.Bacc("TRN2", target_bir_lowering=False, debug=False,
                   num_devices=N_CORES)

    xt = nc.dram_tensor("xt", [128, MT, IT, 128], BF16, kind="ExternalInput")
    xab = nc.dram_tensor("xab", [128, NPAIR, AB_IT // 2, 512], BF16,
                         kind="ExternalInput")
    codes = nc.dram_tensor("codes", [I, O_SH], F16, kind="ExternalInput")
    scales = nc.dram_tensor("scales", [I, O_SH], F16, kind="ExternalInput")
    la = nc.dram_tensor("la", [R, I], BF16, kind="ExternalInput")
    lb = nc.dram_tensor("lb", [R, O_SH], BF16, kind="ExternalInput")
    out = nc.dram_tensor("out", [M, O_SH], F32, kind="ExternalOutput")

    codes_r = codes.ap().rearrange("(t p) o -> p t o", p=128)
    scales_r = scales.ap().rearrange("(t p) o -> p t o", p=128)
    mranges = _macro_ranges()

    with tile.TileContext(nc) as tc:
        with (
            tc.tile_pool(name="wpool", bufs=1) as wpool,
            tc.tile_pool(name="wlab", bufs=5) as wlab,
            tc.tile_pool(name="wlc", bufs=2) as wlc,
            tc.tile_pool(name="dqio", bufs=2) as dqio,
            tc.tile_pool(name="dq", bufs=2) as dq,
            tc.tile_pool(name="xpool", bufs=3) as xpool,
            tc.tile_pool(name="cpool", bufs=1) as cpool,
            tc.tile_pool(name="opool", bufs=3) as opool,
            tc.tile_pool(name="ps", bufs=4, space="PSUM") as pp,
        ):
            # ---- constants ----
            la_sb = cpool.tile([R, I], BF16, tag="la")
            nc.gpsimd.dma_start(la_sb[:], la.ap())
            lb_sb = cpool.tile([R, O_SH], BF16, tag="lb")
            nc.gpsimd.dma_start(lb_sb[:], lb.ap())
            for _ in range(0, N_WARM, 2):
                pl = pp.tile([128, 2 * O_SH], F32, tag="po")
                for h in range(2):
                    nc.tensor.matmul(
                        pl[:, h * O_SH:(h + 1) * O_SH],
                        la_sb[:, 0:128], la_sb[:, 0:O_SH],
                        start=True, stop=True,
                    )
            biases = cpool.tile([128, N_BIAS], F32, tag="bias")
            for j, (g, v) in enumerate(CH7["ramps"] + CH5["ramps"]):
                nc.vector.memset(biases[:, j:j + 1], -abs(g) * v)
            # SBUF bf16 partial accumulator [128, MT*512]
            pa = cpool.tile([128, MT * O_SH], BF16, tag="pa")

            # ---- wl (lora fold) + dummy warm-up on the PE ----
            wl_tiles = {}

            def emit_wl(mi):
                it_lo, it_hi = mranges[mi]
                nt = it_hi - it_lo
                pool = wlab if mi < 5 else wlc
                wl = pool.tile([128, nt * O_SH], F16, tag="wl")
                j = 0
                cnt = 0
                while j < nt:
                    k = min(2, nt - j)
                    pl = pp.tile([128, 2 * O_SH], F32, tag="po")
                    for h in range(k):
                        it = it_lo + j + h
                        nc.tensor.matmul(
                            pl[:, h * O_SH:(h + 1) * O_SH],
                            la_sb[:, it * 128:(it + 1) * 128], lb_sb[:],
                            start=True, stop=True,
                        )
                    dst = wl[:, j * O_SH:(j + k) * O_SH]
                    if cnt % 2 == 0:
                        nc.scalar.copy(dst, pl[:, :k * O_SH])
                    else:
                        nc.vector.tensor_copy(dst, pl[:, :k * O_SH])
                    cnt += 1
                    j += k
                wl_tiles[mi] = wl

            # ---- dequant ----
            w_aps = {}
            slots = {}

            def emit_macro_dma(mi):
                it_lo, it_hi = mranges[mi]
                nt = it_hi - it_lo
                fd = nt * O_SH
                ct = dqio.tile([128, fd], F16, tag="ct")
                nc.gpsimd.dma_start(
                    ct[:].rearrange("p (t o) -> p t o", t=nt),
                    codes_r[:, it_lo:it_hi, :],
                )
                st = dqio.tile([128, fd], F16, tag="st")
                nc.gpsimd.dma_start(
                    st[:].rearrange("p (t o) -> p t o", t=nt),
                    scales_r[:, it_lo:it_hi, :],
                )
                slots[mi] = (ct, st)

            def chain_ops(mi):
                it_lo, it_hi = mranges[mi]
                nt = it_hi - it_lo
                fd = nt * O_SH
                tail_eng = nc.gpsimd if mi >= GP_TAIL_FROM else nc.vector
                ch = CH5 if mi < 5 else CH7
                boff = len(CH7["ramps"]) if mi < 5 else 0
                state = {}
                ops = []

                def op_lin():
                    ct, _ = slots[mi]
                    acc = dq.tile([128, fd], F16, tag="acc")
                    nc.vector.tensor_scalar(
                        acc[:], ct[:], ch["b"], ch["a"],
                        op0=ALU.mult, op1=ALU.add)
                    state["acc"] = acc
                ops.append(op_lin)
                for j, (g, v) in enumerate(ch["ramps"]):
                    def op_ramp(j=j, g=g):
                        ct, _ = slots[mi]
                        r = dq.tile([128, fd], F16, tag="rmp")
                        nc.scalar.activation(
                            r[:], ct[:], ACTF.Relu,
                            bias=biases[:, boff + j:boff + j + 1],
                            scale=abs(g))
                        state["r"] = r
                    ops.append(op_ramp)

                    def op_comb(g=g):
                        acc = state["acc"]
                        nc.vector.tensor_tensor(
                            acc[:], acc[:], state["r"][:],
                            op=ALU.add if g > 0 else ALU.subtract)
                    ops.append(op_comb)

                def op_step():
                    ct, _ = slots[mi]
                    stp = dq.tile([128, fd], F16, tag="rmp")
                    nc.vector.tensor_scalar(
                        stp[:], ct[:], ch["u"], ch["d"],
                        op0=ALU.is_ge, op1=ALU.mult)
                    state["stp"] = stp
                ops.append(op_step)

                def op_addstep():
                    acc = state["acc"]
                    tail_eng.tensor_tensor(
                        acc[:], acc[:], state["stp"][:], op=ALU.add)
                ops.append(op_addstep)

                def op_scale():
                    _, st = slots[mi]
                    acc = state["acc"]
                    tail_eng.tensor_tensor(acc[:], acc[:], st[:], op=ALU.mult)
                ops.append(op_scale)

                def op_lora():
                    wt = wpool.tile([128, fd], BF16, tag=f"w{mi}")
                    tail_eng.tensor_tensor(
                        wt[:], state["acc"][:], wl_tiles[mi][:], op=ALU.add)
                    for j, it in enumerate(range(it_lo, it_hi)):
                        w_aps[it] = wt[:, j * O_SH:(j + 1) * O_SH]
                ops.append(op_lora)
                return ops

            def dma_op(mi):
                return [lambda: emit_macro_dma(mi)]

            pending = []

            def pump(n):
                for _ in range(n):
                    if pending:
                        pending.pop(0)()

            # phase-1 macros fully upfront: m0 gates the first matmul, the
            # rest land chunk-by-chunk just ahead of block 0's consumption
            emit_macro_dma(0)
            emit_macro_dma(1)
            emit_macro_dma(2)
            emit_wl(0)
            pending += chain_ops(0)
            pump(len(pending))
            emit_wl(1)
            emit_wl(2)
            pending += chain_ops(1) + dma_op(3) + chain_ops(2) + dma_op(4)
            pump(len(pending))
            emit_wl(3)
            emit_wl(4)
            pending += chain_ops(3) + dma_op(5) + chain_ops(4)
            pump(len(pending))
            # phase-2 macros pumped through the block loop, dma one ahead
            pending += chain_ops(5) + dma_op(6) + chain_ops(6)
            pending += dma_op(7) + chain_ops(7) + dma_op(8) + chain_ops(8)
            pending += dma_op(9) + chain_ops(9) + dma_op(10) + chain_ops(10)

            # ---- phase 1: M-blocked over i-tiles 0..AB_IT ----
            n_sub = AB_IT // 2
            for blk in range(NBLK):
                po_blk = []
                for sub in range(n_sub):
                    for pr_in in range(4):
                        pr = blk * 4 + pr_in
                        if sub == 0:
                            po_blk.append(pp.tile(
                                [128, 2 * O_SH], F32, tag="po",
                                name=f"po_b{blk}_{pr_in}"))
                        po = po_blk[pr_in]
                        xa = xpool.tile([128, 512], BF16, tag="xab",
                                        bufs=8)
                        nc.sync.dma_start(xa[:], xab.ap()[:, pr, sub, :])
                        for h in range(2):
                            sub_po = po[:, h * O_SH:(h + 1) * O_SH]
                            for k in range(2):
                                off = (h * 2 + k) * 128
                                nc.tensor.matmul(
                                    sub_po, xa[:, off:off + 128],
                                    w_aps[2 * sub + k],
                                    start=(sub == 0 and k == 0),
                                    stop=(sub == n_sub - 1 and k == 1),
                                )
                    if sub < 4:
                        pump(3)
                for pr_in in range(4):
                    pr = blk * 4 + pr_in
                    nc.scalar.copy(
                        pa[:, pr * 2 * O_SH:(pr + 1) * 2 * O_SH],
                        po_blk[pr_in][:])
                if blk < 6:
                    emit_wl(5 + blk)
            pump(len(pending))

            # ---- phase 2: i-tiles AB_IT..32, straight m-loop ----
            n_it = IT - AB_IT
            for pr in range(NPAIR):
                po = pp.tile([128, 2 * O_SH], F32, tag="po")
                for h in range(2):
                    mt = 2 * pr + h
                    xa = xpool.tile([128, n_it, 128], BF16, tag="xc", bufs=3)
                    nc.sync.dma_start(xa[:], xt.ap()[:, mt, AB_IT:, :])
                    sub_po = po[:, h * O_SH:(h + 1) * O_SH]
                    for k in range(n_it):
                        nc.tensor.matmul(
                            sub_po, xa[:, k, :], w_aps[AB_IT + k],
                            start=(k == 0), stop=(k == n_it - 1),
                        )
                pslice = pa[:, pr * 2 * O_SH:(pr + 1) * 2 * O_SH]
                ev = opool.tile([128, 2 * O_SH], F32, tag="ev")
                nc.vector.tensor_tensor(ev[:], po[:], pslice, op=ALU.add)
                dst = out.ap()[pr * 256:(pr + 1) * 256, :]
                nc.scalar.dma_start(
                    dst.rearrange("(b p) o -> p b o", b=2),
                    ev[:].rearrange("p (b o) -> p b o", b=2))

    nc.compile()
    return nc


_NC_CACHE = {}


def _get_nc():
    if "nc" not in _NC_CACHE:
        _NC_CACHE["nc"] = _build_nc()
    return _NC_CACHE["nc"]


def prepare_in_maps(x, w_codes, w_scales, lora_a, lora_b):
    """Host-side sharding + layout prep (no arithmetic beyond casts/folds)."""
    xm = np.ascontiguousarray(x.reshape(M, I))
    # xt[p, mt, t, mm] = x[mt*128+mm, t*128+p], bf16
    xtl = (
        xm.T.reshape(IT, 128, MT, 128)
        .transpose(1, 2, 0, 3)
        .astype(BF16_NP)
    )
    xtl = np.ascontiguousarray(xtl)
    # block-major layout for phase 1: xab[p, pr, sub, (h k mm)]
    nsub = AB_IT // 2
    xabl = (
        xtl[:, :, :AB_IT, :]
        .reshape(128, NPAIR, 2, nsub, 2, 128)
        .transpose(0, 1, 3, 2, 4, 5)
        .reshape(128, NPAIR, nsub, 512)
    )
    xabl = np.ascontiguousarray(xabl)

    la = np.ascontiguousarray(
        (LORA_SCALE * lora_a.astype(np.float64)).astype(BF16_NP)
    )

    in_maps = []
    for c in range(N_CORES):
        o_lo, o_hi = c * O_SH, (c + 1) * O_SH
        codes_t = np.ascontiguousarray(
            w_codes[o_lo:o_hi].T.astype(np.float16)
        )
        scales_t = np.ascontiguousarray(
            np.repeat(w_scales[o_lo:o_hi].T, BLK, axis=0).astype(np.float16)
        )
        lb_t = np.ascontiguousarray(lora_b[o_lo:o_hi].T.astype(BF16_NP))
        in_maps.append(
            {
                "xt": xtl,
                "xab": xabl,
                "codes": codes_t,
                "scales": scales_t,
                "la": la,
                "lb": lb_t,
            }
        )
    return in_maps


def run(in_maps, trace=False, retries=2):
    nc = _get_nc()
    last = None
    for attempt in range(retries + 1):
        try:
            return run_bass_kernel_spmd(
                nc, in_maps, core_ids=list(range(N_CORES)), trace=trace
            )
        except Exception as e:  # transient NRT/axon device errors
            last = e
            if attempt == retries:
                raise
            import time as _time

            _time.sleep(5)
    raise last


def kernel(x, w_codes, w_scales, lora_a, lora_b):
    in_maps = prepare_in_maps(x, w_codes, w_scales, lora_a, lora_b)
    res = run(in_maps, trace=False)
    out = np.concatenate(
        [res.results[c]["out"] for c in range(N_CORES)], axis=1
    )
    return out.reshape(B, S, O).astype(np.float32)


# revision 28
# speedup vs baseline: 1.0618x; 1.0093x over previous
"""NF4-quantized LoRA linear layer on 8 Trainium2 NeuronCores.

Computation (reference):
    w = NF4_TABLE[w_codes] * w_scales[block-expanded]        # [O, I]
    out = x @ w.T + (alpha/rank) * (x @ lora_a.T) @ lora_b.T # [B, S, O]

Strategy:
  - Tensor-parallel split of the output dim across 8 cores (O_SH = 512 each).
    Every core sees all of x; no collectives; host concatenates outputs.
  - LoRA folded into the weights per i-tile on the PE (la.T @ lb); those
    matmuls plus a dummy burst keep the PE busy from t=0 so the HAM clock
    gate is fully open (2.4 GHz) before the real matmuls start.
  - NF4 dequant: 7-term approximate chain (f16 table err 1.5e-3, end-to-end
    err ~3.5e-3 vs the 2e-2 gate): linear+step on DVE tensor_scalar, 5 relu
    ramps on ACT, 6 DVE combines, then *scales and +lora.
  - m-loop phase 1 covers i-tiles 0-11 with M BLOCKED 8-wide: each block of
    8 m-tiles keeps its 4 psum pair-tiles open across six 2-i-tile chunks,
    consuming dequant output just-in-time.  No mid-phase partial evacuation
    or re-add exists at all; each block is evacuated once to a bf16 SBUF
    partial.  Phase 2 (i-tiles 12-31) streams the remaining contraction and
    adds the partial on evacuation.
  - Dequant DMA/compute for later macros is pumped through the block loop
    so no engine FIFO head-blocks; block evacuations live on ACT only,
    phase-2 evacuation adds on DVE, output DMA on the scalar queue.
"""

import numpy as np
import ml_dtypes

import concourse.mybir as mybir
import concourse.tile as tile
from concourse import bacc
from concourse.bass_utils import run_bass_kernel_spmd

B, S, I, O, R, BLK = 4, 2048, 4096, 4096, 16, 64
M = B * S                      # 8192 token rows
N_CORES = 8
O_SH = O // N_CORES            # 512 output cols per core
IT = I // 128                  # 32 contraction tiles
MT = M // 128                  # 64 row tiles
NPAIR = MT // 2                # 32 psum pair-tiles per phase
LORA_SCALE = 2.0               # alpha / rank

# dequant macros: six 2-i-tile chunks feed phase 1 just-in-time, then five
# 4-i-tile macros for phase 2
MACROS = [2, 2, 2, 2, 2, 4, 4, 4, 4, 4, 2]
AB_IT = 10                     # i-tiles covered by phase 1 (macros 0-4)
GP_TAIL_FROM = 5               # macros >= this run the chain tail on GPSIMD
N_WARM = 20                    # dummy warm-up matmuls
NBLK = 8                       # m-blocks in phase 1 (8 m-tiles each)

# NF4 chains: t(c) ~= a + b*c + sum_j g_j*relu(c - v_j) + d*[c>=13.5]
# CH7: 5 ramps, f16 table err 1.45e-3 -- used for phase-2 macros.
# CH5: 3 ramps, f16 table err 9.3e-3 -- used for the startup-critical
# phase-1 macros (12/32 of the contraction; total output err ~6.5e-3
# vs the 2e-2 gate).
CH7 = dict(
    a=-0.999999999931126, b=0.30380719879504586,
    ramps=[
        (-0.020466375583308025, 5.430439005271603),
        (-0.19875188932350388, 1.332340375636196),
        (0.19234278681570927, 12.806559953673979),
    ],
    steps=[
        (0.02775397628252131, 2.5),
        (0.02201050837478022, 11.5),
        (-0.11670333152132338, 13.5),
    ])
CH5 = dict(
    a=-1.0000000000315237, b=0.3038071989637578,
    ramps=[
        (0.18864440149390185, 12.775812349363168),
        (-0.041756800433337744, 3.7380006069052687),
        (-0.17365163624795468, 1.2386672442106303),
    ],
    steps=[(-0.11670333147042945, 13.5)])
N_BIAS = len(CH7["ramps"]) + len(CH5["ramps"])

F16 = mybir.dt.float16
BF16 = mybir.dt.bfloat16
F32 = mybir.dt.float32
ALU = mybir.AluOpType
ACTF = mybir.ActivationFunctionType

BF16_NP = ml_dtypes.bfloat16


def _macro_ranges():
    out, lo = [], 0
    for n in MACROS:
        out.append((lo, lo + n))
        lo += n
    return out


def _build_nc():
    nc = bacc.Bacc("TRN2", target_bir_lowering=False, debug=False,
                   num_devices=N_CORES)

    xt = nc.dram_tensor("xt", [128, MT, IT, 128], BF16, kind="ExternalInput")
    xab = nc.dram_tensor("xab", [128, NPAIR, AB_IT // 2, 512], BF16,
                         kind="ExternalInput")
    codes = nc.dram_tensor("codes", [I, O_SH], F16, kind="ExternalInput")
    scales = nc.dram_tensor("scales", [I, O_SH], F16, kind="ExternalInput")
    la = nc.dram_tensor("la", [R, I], BF16, kind="ExternalInput")
    lb = nc.dram_tensor("lb", [R, O_SH], BF16, kind="ExternalInput")
    out = nc.dram_tensor("out", [M, O_SH], F32, kind="ExternalOutput")

    codes_r = codes.ap().rearrange("(t p) o -> p t o", p=128)
    scales_r = scales.ap().rearrange("(t p) o -> p t o", p=128)
    mranges = _macro_ranges()

    with tile.TileContext(nc) as tc:
        with (
            tc.tile_pool(name="wpool", bufs=1) as wpool,
            tc.tile_pool(name="wlab", bufs=5) as wlab,
            tc.tile_pool(name="wlc", bufs=2) as wlc,
            tc.tile_pool(name="dqio", bufs=2) as dqio,
            tc.tile_pool(name="dq", bufs=2) as dq,
            tc.tile_pool(name="xpool", bufs=3) as xpool,
            tc.tile_pool(name="cpool", bufs=1) as cpool,
            tc.tile_pool(name="opool", bufs=3) as opool,
            tc.tile_pool(name="ps", bufs=4, space="PSUM") as pp,
        ):
            # ---- constants ----
            la_sb = cpool.tile([R, I], BF16, tag="la")
            nc.gpsimd.dma_start(la_sb[:], la.ap())
            lb_sb = cpool.tile([R, O_SH], BF16, tag="lb")
            nc.gpsimd.dma_start(lb_sb[:], lb.ap())
            for _ in range(0, N_WARM, 2):
                pl = pp.tile([128, 2 * O_SH], F32, tag="po")
                for h in range(2):
                    nc.tensor.matmul(
                        pl[:, h * O_SH:(h + 1) * O_SH],
                        la_sb[:, 0:128], la_sb[:, 0:O_SH],
                        start=True, stop=True,
                    )
            biases = cpool.tile([128, N_BIAS], F32, tag="bias")
            for j, (g, v) in enumerate(CH7["ramps"] + CH5["ramps"]):
                nc.vector.memset(biases[:, j:j + 1], -abs(g) * v)
            # SBUF bf16 partial accumulator [128, MT*512]
            pa = cpool.tile([128, MT * O_SH], BF16, tag="pa")

            # ---- wl (lora fold) + dummy warm-up on the PE ----
            wl_tiles = {}

            def emit_wl(mi):
                it_lo, it_hi = mranges[mi]
                nt = it_hi - it_lo
                pool = wlab if mi < 5 else wlc
                wl = pool.tile([128, nt * O_SH], F16, tag="wl")
                j = 0
                cnt = 0
                while j < nt:
                    k = min(2, nt - j)
                    pl = pp.tile([128, 2 * O_SH], F32, tag="po")
                    for h in range(k):
                        it = it_lo + j + h
                        nc.tensor.matmul(
                            pl[:, h * O_SH:(h + 1) * O_SH],
                            la_sb[:, it * 128:(it + 1) * 128], lb_sb[:],
                            start=True, stop=True,
                        )
                    dst = wl[:, j * O_SH:(j + k) * O_SH]
                    if cnt % 2 == 0:
                        nc.scalar.copy(dst, pl[:, :k * O_SH])
                    else:
                        nc.vector.tensor_copy(dst, pl[:, :k * O_SH])
                    cnt += 1
                    j += k
                wl_tiles[mi] = wl

            # ---- dequant ----
            w_aps = {}
            slots = {}

            def emit_macro_dma(mi):
                it_lo, it_hi = mranges[mi]
                nt = it_hi - it_lo
                fd = nt * O_SH
                ct = dqio.tile([128, fd], F16, tag="ct")
                nc.gpsimd.dma_start(
                    ct[:].rearrange("p (t o) -> p t o", t=nt),
                    codes_r[:, it_lo:it_hi, :],
                )
                st = dqio.tile([128, fd], F16, tag="st")
                nc.gpsimd.dma_start(
                    st[:].rearrange("p (t o) -> p t o", t=nt),
                    scales_r[:, it_lo:it_hi, :],
                )
                slots[mi] = (ct, st)

            def chain_ops(mi):
                it_lo, it_hi = mranges[mi]
                nt = it_hi - it_lo
                fd = nt * O_SH
                tail_eng = nc.gpsimd if mi >= GP_TAIL_FROM else nc.vector
                ch = CH5 if mi < 5 else CH7
                boff = len(CH7["ramps"]) if mi < 5 else 0
                state = {}
                ops = []

                def op_lin():
                    ct, _ = slots[mi]
                    acc = dq.tile([128, fd], F16, tag="acc")
                    nc.vector.tensor_scalar(
                        acc[:], ct[:], ch["b"], ch["a"],
                        op0=ALU.mult, op1=ALU.add)
                    state["acc"] = acc
                ops.append(op_lin)
                for j, (g, v) in enumerate(ch["ramps"]):
                    def op_ramp(j=j, g=g):
                        ct, _ = slots[mi]
                        r = dq.tile([128, fd], F16, tag="rmp")
                        nc.scalar.activation(
                            r[:], ct[:], ACTF.Relu,
                            bias=biases[:, boff + j:boff + j + 1],
                            scale=abs(g))
                        state["r"] = r
                    ops.append(op_ramp)

                    def op_comb(g=g):
                        acc = state["acc"]
                        nc.vector.tensor_tensor(
                            acc[:], acc[:], state["r"][:],
                            op=ALU.add if g > 0 else ALU.subtract)
                    ops.append(op_comb)

                for si, (d, u) in enumerate(ch["steps"]):
                    last = si == len(ch["steps"]) - 1

                    def op_step(d=d, u=u):
                        ct, _ = slots[mi]
                        stp = dq.tile([128, fd], F16, tag="rmp")
                        nc.vector.tensor_scalar(
                            stp[:], ct[:], u, d,
                            op0=ALU.is_ge, op1=ALU.mult)
                        state["stp"] = stp
                    ops.append(op_step)

                    def op_addstep(last=last):
                        acc = state["acc"]
                        eng = tail_eng if last else nc.vector
                        eng.tensor_tensor(
                            acc[:], acc[:], state["stp"][:], op=ALU.add)
                    ops.append(op_addstep)

                def op_scale():
                    _, st = slots[mi]
                    acc = state["acc"]
                    tail_eng.tensor_tensor(acc[:], acc[:], st[:], op=ALU.mult)
                ops.append(op_scale)

                def op_lora():
                    wt = wpool.tile([128, fd], BF16, tag=f"w{mi}")
                    tail_eng.tensor_tensor(
                        wt[:], state["acc"][:], wl_tiles[mi][:], op=ALU.add)
                    for j, it in enumerate(range(it_lo, it_hi)):
                        w_aps[it] = wt[:, j * O_SH:(j + 1) * O_SH]
                ops.append(op_lora)
                return ops

            def dma_op(mi):
                return [lambda: emit_macro_dma(mi)]

            pending = []

            def pump(n):
                for _ in range(n):
                    if pending:
                        pending.pop(0)()

            # phase-1 macros fully upfront: m0 gates the first matmul, the
            # rest land chunk-by-chunk just ahead of block 0's consumption
            emit_macro_dma(0)
            emit_macro_dma(1)
            emit_macro_dma(2)
            emit_wl(0)
            pending += chain_ops(0)
            pump(len(pending))
            emit_wl(1)
            emit_wl(2)
            pending += chain_ops(1) + dma_op(3) + chain_ops(2) + dma_op(4)
            pump(len(pending))
            emit_wl(3)
            emit_wl(4)
            pending += chain_ops(3) + dma_op(5) + chain_ops(4)
            pump(len(pending))
            # phase-2 macros pumped through the block loop, dma one ahead
            pending += chain_ops(5) + dma_op(6) + chain_ops(6)
            pending += dma_op(7) + chain_ops(7) + dma_op(8) + chain_ops(8)
            pending += dma_op(9) + chain_ops(9) + dma_op(10) + chain_ops(10)

            # ---- phase 1: M-blocked over i-tiles 0..AB_IT ----
            n_sub = AB_IT // 2
            for blk in range(NBLK):
                po_blk = []
                for sub in range(n_sub):
                    for pr_in in range(4):
                        pr = blk * 4 + pr_in
                        if sub == 0:
                            po_blk.append(pp.tile(
                                [128, 2 * O_SH], F32, tag="po",
                                name=f"po_b{blk}_{pr_in}"))
                        po = po_blk[pr_in]
                        xa = xpool.tile([128, 512], BF16, tag="xab",
                                        bufs=8)
                        nc.sync.dma_start(xa[:], xab.ap()[:, pr, sub, :])
                        for h in range(2):
                            sub_po = po[:, h * O_SH:(h + 1) * O_SH]
                            for k in range(2):
                                off = (h * 2 + k) * 128
                                nc.tensor.matmul(
                                    sub_po, xa[:, off:off + 128],
                                    w_aps[2 * sub + k],
                                    start=(sub == 0 and k == 0),
                                    stop=(sub == n_sub - 1 and k == 1),
                                )
                    if sub < 4:
                        pump(3)
                for pr_in in range(4):
                    pr = blk * 4 + pr_in
                    nc.scalar.copy(
                        pa[:, pr * 2 * O_SH:(pr + 1) * 2 * O_SH],
                        po_blk[pr_in][:])
                if blk < 6:
                    emit_wl(5 + blk)
            pump(len(pending))

            # ---- phase 2: i-tiles AB_IT..32, straight m-loop ----
            n_it = IT - AB_IT
            for pr in range(NPAIR):
                po = pp.tile([128, 2 * O_SH], F32, tag="po")
                for h in range(2):
                    mt = 2 * pr + h
                    xa = xpool.tile([128, n_it, 128], BF16, tag="xc", bufs=3)
                    nc.sync.dma_start(xa[:], xt.ap()[:, mt, AB_IT:, :])
                    sub_po = po[:, h * O_SH:(h + 1) * O_SH]
                    for k in range(n_it):
                        nc.tensor.matmul(
                            sub_po, xa[:, k, :], w_aps[AB_IT + k],
                            start=(k == 0), stop=(k == n_it - 1),
                        )
                pslice = pa[:, pr * 2 * O_SH:(pr + 1) * 2 * O_SH]
                ev = opool.tile([128, 2 * O_SH], F32, tag="ev")
                nc.vector.tensor_tensor(ev[:], po[:], pslice, op=ALU.add)
                dst = out.ap()[pr * 256:(pr + 1) * 256, :]
                nc.scalar.dma_start(
                    dst.rearrange("(b p) o -> p b o", b=2),
                    ev[:].rearrange("p (b o) -> p b o", b=2))

    nc.compile()
    return nc


_NC_CACHE = {}


def _get_nc():
    if "nc" not in _NC_CACHE:
        _NC_CACHE["nc"] = _build_nc()
    return _NC_CACHE["nc"]


def prepare_in_maps(x, w_codes, w_scales, lora_a, lora_b):
    """Host-side sharding + layout prep (no arithmetic beyond casts/folds)."""
    xm = np.ascontiguousarray(x.reshape(M, I))
    # xt[p, mt, t, mm] = x[mt*128+mm, t*128+p], bf16
    xtl = (
        xm.T.reshape(IT, 128, MT, 128)
        .transpose(1, 2, 0, 3)
        .astype(BF16_NP)
    )
    xtl = np.ascontiguousarray(xtl)
    # block-major layout for phase 1: xab[p, pr, sub, (h k mm)]
    nsub = AB_IT // 2
    xabl = (
        xtl[:, :, :AB_IT, :]
        .reshape(128, NPAIR, 2, nsub, 2, 128)
        .transpose(0, 1, 3, 2, 4, 5)
        .reshape(128, NPAIR, nsub, 512)
    )
    xabl = np.ascontiguousarray(xabl)

    la = np.ascontiguousarray(
        (LORA_SCALE * lora_a.astype(np.float64)).astype(BF16_NP)
    )

    in_maps = []
    for c in range(N_CORES):
        o_lo, o_hi = c * O_SH, (c + 1) * O_SH
        codes_t = np.ascontiguousarray(
            w_codes[o_lo:o_hi].T.astype(np.float16)
        )
        scales_t = np.ascontiguousarray(
            np.repeat(w_scales[o_lo:o_hi].T, BLK, axis=0).astype(np.float16)
        )
        lb_t = np.ascontiguousarray(lora_b[o_lo:o_hi].T.astype(BF16_NP))
        in_maps.append(
            {
                "xt": xtl,
                "xab": xabl,
                "codes": codes_t,
                "scales": scales_t,
                "la": la,
                "lb": lb_t,
            }
        )
    return in_maps


def run(in_maps, trace=False, retries=2):
    nc = _get_nc()
    last = None
    for attempt in range(retries + 1):
        try:
            return run_bass_kernel_spmd(
                nc, in_maps, core_ids=list(range(N_CORES)), trace=trace
            )
        except Exception as e:  # transient NRT/axon device errors
            last = e
            if attempt == retries:
                raise
            import time as _time

            _time.sleep(5)
    raise last


def kernel(x, w_codes, w_scales, lora_a, lora_b):
    in_maps = prepare_in_maps(x, w_codes, w_scales, lora_a, lora_b)
    res = run(in_maps, trace=False)
    out = np.concatenate(
        [res.results[c]["out"] for c in range(N_CORES)], axis=1
    )
    return out.reshape(B, S, O).astype(np.float32)
